# revision 26
# baseline (speedup 1.0000x reference)
"""GRU-D style GRUI encoder kernel for Trainium2 (Bass/Tile), 8 NeuronCores.

Data-parallel over batch B=256 across 8 cores (BL=32 sequences/core).
Layout: hidden-on-partitions (H=256 -> 2 k-tiles of 128), batch-on-free.

The T=512 recurrence is latency-bound: makespan ~= T * L where L is the
per-step cross-engine dependency chain.  This kernel minimizes L to 6 hops:

    tanh(z_h) -> y(DVE) -> y-matmuls(PE) -> tanh(z_r/2)(Act) -> rh(DVE)
      -> h-matmuls(PE) -> tanh(z_h) ...

Key tricks:
 - All gates use Tanh only (sigmoid(z) = 0.5*tanh(z/2)+0.5), so Tanh+Exp
   live in one activation table -> zero act-table switches.
 - State split hb(t+1) = q(t) + y(t) with q = beta'(.)hb - w(.)hb ready
   early (off-path) and y = w(.)hhat; matmul linearity gives
   W.hb(t+1) = W.q + W.y, so only the y-matmuls follow tanh(z_h).
 - Precompute (x-GEMMs, biases) writes directly into the recurrence PSUM
   accumulation groups (4 steps/group); biases are injected via tiny
   block-indicator matmuls.  No injection matmuls, no PSUM->SBUF copies,
   spread over the previous group's iterations to avoid PE bursts.
 - beta = min(exp(-(Wtd.delta + b)), 1) inline per 16 steps, with its
   Act/PE/Pool pieces placed in per-iteration slack windows.
"""

import numpy as np
import ml_dtypes
from contextlib import ExitStack

import concourse.bass as bass
from bass_rust import InstructionNameOrderedSet
import concourse.bacc as bacc
import concourse.tile as tile
from concourse import mybir
from concourse.bass_utils import run_bass_kernel_spmd

B, T, D, H = 256, 512, 128, 256
NCORES = 8
BL = B // NCORES          # 32 sequences per core
C = 64                    # x/delta DMA chunk (steps)
G4 = 4                    # psum accumulation group (steps)
G16 = 8                   # beta group (steps)

FP32 = mybir.dt.float32
BF16 = mybir.dt.bfloat16
AF = mybir.ActivationFunctionType
OP = mybir.AluOpType

_cache = {}


def _build():
    nc = bacc.Bacc("TRN2", target_bir_lowering=False, debug=False,
                   num_devices=NCORES)

    xT = nc.dram_tensor("xT", [D, T * BL], BF16, kind="ExternalInput")
    dTs = nc.dram_tensor("dTs", [D, T * BL], BF16, kind="ExternalInput")
    wx_rmu_d = nc.dram_tensor("wx_rmu", [D, 4 * 128], BF16, kind="ExternalInput")
    wxh_d = nc.dram_tensor("wxh", [D, 2 * 128], BF16, kind="ExternalInput")
    wtd_d = nc.dram_tensor("wtd", [D, 2 * 128], BF16, kind="ExternalInput")
    whr0_d = nc.dram_tensor("whr0", [128, 4 * 128], BF16, kind="ExternalInput")
    whr1_d = nc.dram_tensor("whr1", [128, 4 * 128], BF16, kind="ExternalInput")
    whh0h_d = nc.dram_tensor("whh0h", [128, 2 * 128], BF16, kind="ExternalInput")
    whh1h_d = nc.dram_tensor("whh1h", [128, 2 * 128], BF16, kind="ExternalInput")
    brmu_d = nc.dram_tensor("brmu_l", [4, 128], BF16, kind="ExternalInput")
    bh_d = nc.dram_tensor("bh_l", [2, 128], BF16, kind="ExternalInput")
    ind4_d = nc.dram_tensor("ind4", [4, G4 * 4 * BL], BF16, kind="ExternalInput")
    ind2_d = nc.dram_tensor("ind2", [2, G4 * 2 * BL], BF16, kind="ExternalInput")
    nbtd_d = nc.dram_tensor("nb_td", [128, 2], FP32, kind="ExternalInput")
    out_d = nc.dram_tensor("hT_out", [128, 2 * BL], FP32, kind="ExternalOutput")

    NCH = T // C
    NG4 = T // G4
    NG16 = T // G16

    with ExitStack() as ctx:
        tc = ctx.enter_context(tile.TileContext(nc))
        wpool = ctx.enter_context(tc.tile_pool(name="weights", bufs=1))
        xpool = ctx.enter_context(tc.tile_pool(name="xin", bufs=2))
        bpool = ctx.enter_context(tc.tile_pool(name="beta", bufs=3))
        prmu = ctx.enter_context(tc.tile_pool(name="prmu", bufs=2, space="PSUM"))
        phpl = ctx.enter_context(tc.tile_pool(name="ph", bufs=2, space="PSUM"))
        pbeta = ctx.enter_context(tc.tile_pool(name="pbeta", bufs=2, space="PSUM"))
        spool = ctx.enter_context(tc.tile_pool(name="state", bufs=3))

        # ---- weights / constants into SBUF ----
        def wtile(shape, dt, dram):
            t_ = wpool.tile(shape, dt, tag=dram.name, name=dram.name)
            nc.sync.dma_start(t_, dram[:, :])
            return t_

        wx_rmu = wtile([128, 512], BF16, wx_rmu_d)
        wxh = wtile([128, 256], BF16, wxh_d)
        wtd = wtile([128, 256], BF16, wtd_d)
        whr = [wtile([128, 512], BF16, whr0_d), wtile([128, 512], BF16, whr1_d)]
        whh = [wtile([128, 256], BF16, whh0h_d), wtile([128, 256], BF16, whh1h_d)]
        brmu = wtile([4, 128], BF16, brmu_d)
        bh = wtile([2, 128], BF16, bh_d)
        ind4 = wtile([4, G4 * 4 * BL], BF16, ind4_d)
        ind2 = wtile([2, G4 * 2 * BL], BF16, ind2_d)
        nbtd = wtile([128, 2], FP32, nbtd_d)

        hb = spool.tile([128, 2 * BL], BF16, tag="hb")   # hb(0) = 0
        nc.vector.memset(hb, 0.0)

        xch = [None] * NCH
        dch = [None] * NCH

        def load_chunk(c):
            xch[c] = xpool.tile([128, C * BL], BF16, tag="xch", name="xch")
            nc.sync.dma_start(xch[c], xT[:, c * C * BL:(c + 1) * C * BL])
            dch[c] = xpool.tile([128, C * BL], BF16, tag="dch", name="dch")
            nc.sync.dma_start(dch[c], dTs[:, c * C * BL:(c + 1) * C * BL])

        # ---- beta (per G16 steps), in phases ----
        beta = [None] * NG16     # SBUF [128, 2, G16*BL] bf16
        bps = {}                 # psum per k

        def beta_mms(g):
            c = (g * G16) // C
            off = (g * G16 * BL) % (C * BL)
            beta[g] = bpool.tile([128, 2, G16 * BL], BF16, tag="beta", name="beta")
            for k in range(2):
                ps = pbeta.tile([128, G16 * BL], FP32, tag=f"bps{k}")
                for j in range(G16):
                    sl = slice(j * 32, (j + 1) * 32)
                    nc.tensor.matmul(ps[:, sl], wtd[:, k * 128:(k + 1) * 128],
                                     dch[c][:, off + j * 32:off + (j + 1) * 32],
                                     start=True, stop=True)
                bps[(g, k)] = ps

        def beta_exp(g, k, half):
            # exp(-(z + b)) = Exp(-z + (-b));  half in (0, 1)
            sl = slice(half * 128, (half + 1) * 128)
            nc.scalar.activation(beta[g][:, k, sl], bps[(g, k)][:, sl], AF.Exp,
                                 bias=nbtd[:, k:k + 1], scale=-1.0)

        def beta_min(g):
            # beta_h = 0.5 * min(exp, 1)   (the 0.5 folds mu = 0.5*(Gm+1))
            # On DVE so downstream TSP reads carry no cross-engine wait.
            nc.vector.tensor_scalar(beta[g], beta[g], 1.0, 0.5, OP.min, OP.mult)

        def beta_ap(t):
            return beta[t // G16][:, :, (t % G16) * BL:(t % G16 + 1) * BL]

        # ---- precompute pieces for psum group g (G4 steps), spread out ----
        # PSUM zero-region semantics: ONE start=True write per group tile
        # (it marks the whole 2KB region pending-zero); every other write
        # accumulates (start=False, first touch of each byte lands as
        # overwrite); ONE stop=True on the region's final write.
        rmu_g = [None] * NG4     # psum [128, G4, 4*BL] fp32
        h_g = [None] * NG4       # psum [128, G4, 2*BL] fp32
        rmu_start = [None] * NG4  # the start=True instruction per region
        h_start = [None] * NG4

        def _dep_on(inst, start_inst):
            d = InstructionNameOrderedSet()
            d.add(start_inst.ins.name)
            inst.ins.add_nosync_dependencies_from(d)

        def pre_rmu_gx(g, ms):
            c = (g * G4) // C
            off = (g * G4 * BL) % (C * BL)
            for m in ms:
                for s in range(G4):
                    first = rmu_start[g] is None
                    inst = nc.tensor.matmul(
                        rmu_g[g][:, s, m * BL:(m + 1) * BL],
                        wx_rmu[:, m * 128:(m + 1) * 128],
                        xch[c][:, off + s * BL:off + (s + 1) * BL],
                        start=first, stop=False, skip_group_check=True)
                    if first:
                        rmu_start[g] = inst
                    else:
                        _dep_on(inst, rmu_start[g])

        def pre_h_gx(g):
            c = (g * G4) // C
            off = (g * G4 * BL) % (C * BL)
            for m in range(2):
                for s in range(G4):
                    first = h_start[g] is None
                    inst = nc.tensor.matmul(
                        h_g[g][:, s, m * BL:(m + 1) * BL],
                        wxh[:, m * 128:(m + 1) * 128],
                        xch[c][:, off + s * BL:off + (s + 1) * BL],
                        start=first, stop=False, skip_group_check=True)
                    if first:
                        h_start[g] = inst
                    else:
                        _dep_on(inst, h_start[g])

        def pre_h_bias(g):
            flat = h_g[g].rearrange("p t c -> p (t c)")
            for j in range(8):
                sl = slice(j * 32, (j + 1) * 32)
                inst = nc.tensor.matmul(flat[:, sl], bh, ind2[:, sl],
                                        start=False, stop=False,
                                        skip_group_check=True)
                _dep_on(inst, h_start[g])

        def pre_rmu_bias(g):
            flat = rmu_g[g].rearrange("p t c -> p (t c)")
            for j in range(16):
                sl = slice(j * 32, (j + 1) * 32)
                inst = nc.tensor.matmul(flat[:, sl], brmu, ind4[:, sl],
                                        start=False, stop=False,
                                        skip_group_check=True)
                _dep_on(inst, rmu_start[g])

        def alloc_group(g):
            rmu_g[g] = prmu.tile([128, G4, 4 * BL], FP32, tag="prmu", name="prmu")
            h_g[g] = phpl.tile([128, G4, 2 * BL], FP32, tag="ph", name="ph")

        # recurrence state-dependent matmuls into slot tau
        def rmu_mms(g, tau, vec, stop, ms=(0, 1, 2, 3)):
            for m in ms:
                for k in range(2):
                    inst = nc.tensor.matmul(
                        rmu_g[g][:, tau, m * BL:(m + 1) * BL],
                        whr[k][:, m * 128:(m + 1) * 128],
                        vec[:, k * BL:(k + 1) * BL],
                        start=False,
                        stop=(stop and m == ms[-1] and k == 1),
                        skip_group_check=True)
                    _dep_on(inst, rmu_start[g])

        def h_mms(g, tau, vec, stop):
            for m in range(2):
                for k in range(2):
                    inst = nc.tensor.matmul(
                        h_g[g][:, tau, m * BL:(m + 1) * BL],
                        whh[k][:, m * 128:(m + 1) * 128],
                        vec[:, k * BL:(k + 1) * BL],
                        start=False, stop=(stop and m == 1 and k == 1),
                        skip_group_check=True)
                    _dep_on(inst, h_start[g])

        # ---- prologue: chunk 0, beta group 0, psum group 0 ----
        load_chunk(0)
        beta_mms(0)
        for _k in range(2):
            for _h in range(2):
                beta_exp(0, _k, _h)
        beta_min(0)
        alloc_group(0)
        pre_rmu_gx(0, (0, 1))
        pre_rmu_gx(0, (2, 3))
        pre_h_gx(0)
        pre_h_bias(0)
        pre_rmu_bias(0)
        # close slot 0 accumulation with zero contributions (hb(0)=0)
        rmu_mms(0, 0, hb, stop=False)
        h_mms(0, 0, hb, stop=False)   # hb-part of z_h for slot 0 (zero)

        q_t = None
        y_t = None

        for t in range(T):
            tau = t % G4
            g = t // G4
            last = (t == T - 1)

            # finished-beta min pass in the DVE idle window at iter start
            if t % G16 == 6 and t // G16 + 1 < NG16:
                with tc.high_priority(offset=-10**6):
                    beta_min(t // G16 + 1)

            # bh2 = -(beta_h (.) hb): independent of this step's gates,
            # lets q be a single op off G_mu (no serial DVE chain)
            if not last:
                with tc.high_priority():
                    bh2 = spool.tile([128, 2 * BL], BF16, tag="bh2", name="bh2")
                    nc.vector.scalar_tensor_tensor(bh2, hb, -1.0, beta_ap(t),
                                                   OP.mult, OP.mult)

            # ---- activations for step t ----
            Gr = spool.tile([128, 2 * BL], BF16, tag="Gr")
            nc.scalar.activation(Gr, rmu_g[g][:, tau, 0:2 * BL], AF.Tanh,
                                 scale=0.5)
            Gm = spool.tile([128, 2 * BL], BF16, tag="Gm")
            nc.scalar.activation(Gm, rmu_g[g][:, tau, 2 * BL:4 * BL], AF.Tanh,
                                 scale=0.5)

            # rh' = G_r (.) hb   (the "+1" half went in via early h-mms)
            rh = spool.tile([128, 2 * BL], BF16, tag="rh")
            nc.vector.tensor_mul(rh, Gr, hb)

            # off-path update pieces: w = mu*beta', q = (1-mu)*beta'(.)hb
            # (beta tile is pre-halved: beta_h = 0.5*beta')
            if not last:
                bap = beta_ap(t)
                with tc.high_priority():
                    # q = (G_mu - 1)(.)bh2 = (1-mu)*beta' (.) hb
                    q_t = spool.tile([128, 2 * BL], BF16, tag="q", name="q")
                    q_inst = nc.vector.scalar_tensor_tensor(q_t, Gm, 1.0, bh2,
                                                            OP.subtract, OP.mult)
                    # w = (G_mu + 1)(.)beta_h = mu*beta'
                    w_ = spool.tile([128, 2 * BL], BF16, tag="w", name="w")
                    nc.vector.scalar_tensor_tensor(w_, Gm, 1.0, bap,
                                                    OP.add, OP.mult)

            h_mms(g, tau, rh, stop=(tau == G4 - 1))

            # tanh(z_h)
            hh = spool.tile([128, 2 * BL], BF16, tag="hh")
            nc.scalar.activation(hh, h_g[g][:, tau, :], AF.Tanh)

            # beta pipeline pieces (in engine slack windows)
            r16 = t % G16
            g16n = t // G16 + 1
            if g16n < NG16 and 1 <= r16 <= 4:
                beta_exp(g16n, (r16 - 1) // 2, (r16 - 1) % 2)

            if last:
                # h_out = hb + 0.5*(G_mu+1) (.) (hh - hb)
                d_ = spool.tile([128, 2 * BL], BF16, tag="d")
                nc.vector.tensor_tensor(d_, hh, hb, op=OP.subtract)
                e_ = spool.tile([128, 2 * BL], BF16, tag="e")
                nc.vector.scalar_tensor_tensor(e_, Gm, 1.0, d_, OP.add, OP.mult)
                hout = spool.tile([128, 2 * BL], FP32, tag="ho")
                nc.vector.scalar_tensor_tensor(hout, e_, 0.5, hb, OP.mult, OP.add)
                nc.sync.dma_start(out_d[:, :], hout)
                break

            # y = w (.) hh   [on critical path]
            y_t = spool.tile([128, 2 * BL], BF16, tag="y")
            nc.vector.tensor_mul(y_t, w_, hh)

            # q/y matmuls for slot t+1; r-blocks of y first (they gate sig_r)
            ntau = (t + 1) % G4
            tg = (t + 1) // G4
            rmu_mms(tg, ntau, q_t, stop=False)
            with tc.high_priority():
                rmu_mms(tg, ntau, y_t, stop=False, ms=(0, 1))
            rmu_mms(tg, ntau, y_t, stop=(ntau == G4 - 1), ms=(2, 3))

            # hb(t+1) = q + y  (feeds rh(t+1), v(t+1), early h-mms(t+1))
            hb = spool.tile([128, 2 * BL], BF16, tag="hb")
            nc.vector.tensor_add(hb, q_t, y_t)

            # early h-mms: Whh_half . hb(t+1)  (the "+1" part of the r gate)
            h_mms(tg, ntau, hb, stop=False)

            # spread next-group precompute across this group's iterations;
            # finish by tau==2 so nothing precompute-side gates sig_r(slot 0).
            # Low priority: the scheduler must prefer ready recurrence work.
            with tc.high_priority(offset=-10**6):
                ng = g + 1
                if ng < NG4:
                    if tau == 0:
                        alloc_group(ng)
                        pre_rmu_gx(ng, (0, 1))
                    elif tau == 1:
                        pre_rmu_gx(ng, (2, 3))
                        pre_h_gx(ng)
                    elif tau == 2:
                        pre_h_bias(ng)
                        pre_rmu_bias(ng)

                # beta matmuls in the PE dead window
                if g16n < NG16 and r16 == 0:
                    beta_mms(g16n)

                # chunk DMA lookahead
                if t % C == C // 2 and t // C + 1 < NCH:
                    load_chunk(t // C + 1)

    nc.compile()
    return nc


def _prep_inputs(x, delta, W_mu, b_mu, W_r, b_r, W_h, b_h, W_td, b_td):
    bf = ml_dtypes.bfloat16
    # weights: first H rows act on h, last D rows act on x
    wh_rmu = np.concatenate([W_r[:H], W_mu[:H]], axis=1)      # [256, 512]
    wx_rmu = np.concatenate([W_r[H:], W_mu[H:]], axis=1)      # [128, 512]
    wh_h, wx_h = W_h[:H], W_h[H:]

    def pcol(v):  # [2*128] -> [128, 2] column-per-tile
        return np.ascontiguousarray(np.stack([v[:128], v[128:]], axis=1),
                                    dtype=np.float32)

    brmu_l = np.ascontiguousarray(
        np.stack([b_r[:128], b_r[128:], b_mu[:128], b_mu[128:]]), dtype=bf)
    bh_l = np.ascontiguousarray(np.stack([b_h[:128], b_h[128:]]), dtype=bf)

    cols4 = np.arange(G4 * 4 * BL)
    ind4 = np.ascontiguousarray(
        (cols4[None, :] // BL % 4 == np.arange(4)[:, None]).astype(bf))
    cols2 = np.arange(G4 * 2 * BL)
    ind2 = np.ascontiguousarray(
        (cols2[None, :] // BL % 2 == np.arange(2)[:, None]).astype(bf))

    shared = {
        "wx_rmu": np.ascontiguousarray(wx_rmu, dtype=bf),
        "wxh": np.ascontiguousarray(wx_h, dtype=bf),
        "wtd": np.ascontiguousarray(W_td, dtype=bf),
        "whr0": np.ascontiguousarray(wh_rmu[:128], dtype=bf),
        "whr1": np.ascontiguousarray(wh_rmu[128:], dtype=bf),
        "whh0h": np.ascontiguousarray(0.5 * wh_h[:128], dtype=bf),
        "whh1h": np.ascontiguousarray(0.5 * wh_h[128:], dtype=bf),
        "brmu_l": brmu_l,
        "bh_l": bh_l,
        "ind4": ind4,
        "ind2": ind2,
        "nb_td": pcol(-b_td),
    }

    # delta shifted by one step: beta used at step t is beta(t+1)
    dshift = np.concatenate(
        [delta[:, 1:, :], np.zeros((B, 1, D), np.float32)], axis=1)

    in_maps = []
    for ci in range(NCORES):
        xs = x[ci * BL:(ci + 1) * BL]          # [32, 512, 128]
        ds = dshift[ci * BL:(ci + 1) * BL]
        # [BL, T, D] -> [D, T, BL] -> [D, T*BL]  (column t*BL + b)
        xt = np.ascontiguousarray(
            xs.transpose(2, 1, 0).reshape(D, T * BL), dtype=bf)
        dt_ = np.ascontiguousarray(
            ds.transpose(2, 1, 0).reshape(D, T * BL), dtype=bf)
        in_maps.append({"xT": xt, "dTs": dt_, **shared})
    return in_maps


def kernel(x, delta, W_mu, b_mu, W_r, b_r, W_h, b_h, W_td, b_td):
    args = tuple(np.asarray(a, dtype=np.float32) for a in
                 (x, delta, W_mu, b_mu, W_r, b_r, W_h, b_h, W_td, b_td))
    in_maps = _prep_inputs(*args)
    if "nc" not in _cache:
        _cache["nc"] = _build()
    res = run_bass_kernel_spmd(_cache["nc"], in_maps,
                               core_ids=list(range(NCORES)))
    out = np.empty((B, H), np.float32)
    for ci in range(NCORES):
        o = res.results[ci]["hT_out"]          # [128, 2*BL]
        for k in range(2):
            # o[p, k*BL + b] = h[b, k*128 + p]
            out[ci * BL:(ci + 1) * BL, k * 128:(k + 1) * 128] = \
                o[:, k * BL:(k + 1) * BL].T
    return out
